# revision 35
# baseline (speedup 1.0000x reference)
"""Trainium2 Bass kernel for nn_ExtendedAnomalyNet (patch-CNN over 24x24 map).

Algorithm: multiPool decomposition — conv1 is shared on the padded image,
the two stride-2 maxpools become 4/16 parity-indexed pooled maps, so
conv2/conv3 run once per parity combination (~25x fewer FLOPs than naive
per-patch evaluation).

Sharding (8 cores): core c = (oy, ox, h): pool-parity (oy, ox) in {0,1}^2
and spatial half h in {0,1}. Every stage after the host-built conv1 im2col
is core-local; each core produces 72 of the 576 output pixels.

Perf notes (28.4µs vs 37µs baseline; all mechanisms trace-verified on HW):
 - Input DMAs: ONE HWDGE ring (sync), strict need-order. Two rings
   round-robin per packet and starve small critical transfers; each
   dma_start also costs a ~0.3-0.8µs ring bubble (write-receipt
   serializes), so transfers are few and large: w1 rides inside r1's
   tensor (75 im2col rows only), w45+dense weights inside w3's.
 - PE warm-up matmuls on a zeroed tile while DMAs stream, plus bridge
   matmuls in the conv1->conv2 and pool2 stalls: HAM un-throttles
   (1.2->2.4 GHz) during conv1 and never re-throttles. Keep every PE
   idle gap under ~1µs — 4 instead of 6 warm-ups measurably lets the
   re-throttle window catch conv2 cold.
 - pools: 2 large tensor_max passes (+4 gathers on vector/gpsimd)
   instead of 12 serialized small ops; pool1 colmax split at row 15 so
   conv2's dy=0 taps start on region deps. Activations stay on the
   Scalar ACT engine (Vector lrelu on fp32 PSUM reads is half-rate).
 - ACT tables preloaded off the critical path; dense bias-adds split
   Scalar/Vector; output fp16.
"""
import numpy as np

IMH = IMW = 24

_CACHE = {}


def _host_prep(x, c1w, c1b, c2w, c2b, c3w, c3b, c4w, c4b, c5w, c5b, dw, db):
    xp = np.pad(np.asarray(x, np.float32)[0], ((0, 0), (16, 16), (16, 16)))  # (3,56,56)
    sw = np.lib.stride_tricks.sliding_window_view(xp, (5, 5), axis=(1, 2))  # (3,52,52,5,5)
    w1 = np.ascontiguousarray(
        np.asarray(c1w, np.float32).reshape(128, 75).T
    ).astype(np.float16)
    r1s = []
    for c in range(8):
        oy, ox, h = (c >> 2) & 1, (c >> 1) & 1, c & 1
        r0, c0 = oy + 12 * h, ox
        r1 = np.empty((75, 38 * 50 + 128), np.float16)
        r1[:, :1900] = (
            sw[:, r0:r0 + 38, c0:c0 + 50, :, :]
            .transpose(0, 3, 4, 1, 2)
            .reshape(75, 38 * 50)
        )
        r1[:, 1900:] = w1  # conv1 weights ride along in the same DMA
        r1s.append(r1)
    w2 = np.ascontiguousarray(
        np.asarray(c2w, np.float32).transpose(2, 3, 1, 0)  # (dy,dx,i,o)
    ).transpose(2, 0, 1, 3).reshape(128, 25 * 128).astype(np.float16)
    w3 = np.ascontiguousarray(
        np.asarray(c3w, np.float32).transpose(2, 3, 1, 0)
    ).transpose(2, 0, 1, 3).reshape(128, 25 * 128).astype(np.float16)
    w45d = np.zeros((128, 8, 128), np.float16)
    c4 = np.asarray(c4w, np.float32)[:, :, 0, 0]
    c5 = np.asarray(c5w, np.float32)[:, :, 0, 0]
    dwf = np.asarray(dw, np.float32)
    w45d[:, 0, :] = c4[:128, :].T
    w45d[:, 1, :] = c4[128:, :].T
    w45d[:, 2, :] = c5[:, :128].T
    w45d[:, 3, :] = c5[:, 128:].T
    for q in range(4):
        w45d[:, 4 + q, :] = dwf[128 * q:128 * (q + 1), :].T
    w3 = np.concatenate([w3, w45d.reshape(128, 1024)], axis=1)  # one DMA
    biases = np.zeros((128, 10), np.float32)
    biases[:, 0] = np.asarray(c1b, np.float32)
    biases[:, 1] = np.asarray(c2b, np.float32)
    biases[:, 2] = np.asarray(c3b, np.float32)
    biases[:, 3] = np.asarray(c4b, np.float32)[:128]
    biases[:, 4] = np.asarray(c4b, np.float32)[128:]
    biases[:, 5] = np.asarray(c5b, np.float32)
    biases[:, 6:10] = np.asarray(db, np.float32).reshape(4, 128).T
    return r1s, w1, w2, w3, w45d.reshape(128, 1024), biases


def _build_nc():
    from contextlib import ExitStack

    import concourse.bass as bass
    import concourse.bacc as bacc
    import concourse.mybir as mybir
    import concourse.tile as tile

    dt = mybir.dt
    AF = mybir.ActivationFunctionType
    ALU = mybir.AluOpType

    nc = bacc.Bacc("TRN2", debug=False, num_devices=8)
    R1 = nc.dram_tensor("r1", [75, 2028], dt.float16, kind="ExternalInput").ap()
    W2 = nc.dram_tensor("w2", [128, 3200], dt.float16, kind="ExternalInput").ap()
    W3 = nc.dram_tensor("w3", [128, 4224], dt.float16, kind="ExternalInput").ap()
    BIAS = nc.dram_tensor("biases", [128, 10], dt.float32, kind="ExternalInput").ap()
    FEATS = nc.dram_tensor("feats", [128, 288], dt.float16, kind="ExternalOutput").ap()

    with tile.TileContext(nc) as tc, ExitStack() as ctx:
        const = ctx.enter_context(tc.tile_pool(name="const", bufs=1))
        work = ctx.enter_context(tc.tile_pool(name="work", bufs=1))
        ps = ctx.enter_context(tc.tile_pool(name="ps", bufs=4, space="PSUM"))
        psw = ctx.enter_context(tc.tile_pool(name="psw", bufs=1, space="PSUM"))

        # --- tiles ---
        r1t = const.tile([75, 2028], dt.float16)
        w1t = r1t[:, 1900:2028]
        bt = const.tile([128, 10], dt.float32)
        w2t = const.tile([128, 25, 128], dt.float16)
        w3t = const.tile([128, 33, 128], dt.float16)
        w45t = w3t[:, 25:33, :]
        warm = const.tile([128, 512], dt.float16)

        # --- PE warm-up: memset a tile, then dependency-free matmuls keep
        # the PE busy while input DMAs stream, so HAM flips to 2.4 GHz
        # during conv1 and conv2 runs warm. Results are never read.
        nc.vector.memset(warm[:], 0.0)
        pw = psw.tile([128, 512], dt.float32, tag="warm")
        # 6 warm-up matmuls: enough that conv1 starts right as they drain and
        # every later PE idle gap stays well under the ~3.4µs HAM re-throttle
        # window (4 warm-ups measurably lets conv2 go cold).
        for _ in range(6):
            nc.tensor.matmul(pw[:], warm[:, 0:128], warm[:], start=True, stop=True)

        # --- input DMAs: ALL on the sync HWDGE ring, in need-order.
        # One ring = strict FIFO = stream bandwidth follows exactly this
        # priority order (two rings round-robin per packet and starve the
        # small critical transfers). Few LARGE transfers: each dma_start
        # costs a ~0.3-0.8µs ring bubble (write-receipt serializes), so w1
        # rides in r1's tensor and w45 in w3's.
        W2v = W2.rearrange("p (t o) -> p t o", t=25)
        nc.sync.dma_start(out=r1t[:], in_=R1)
        nc.sync.dma_start(out=bt[:], in_=BIAS)
        nc.sync.dma_start(out=w2t[:, 0:13, :], in_=W2v[:, 0:13, :])
        nc.sync.dma_start(out=w2t[:, 13:25, :], in_=W2v[:, 13:25, :])
        nc.sync.dma_start(out=w3t[:], in_=W3.rearrange("p (t o) -> p t o", t=33))

        # --- activation-table preload: Lrelu now (needed by conv1 ACT);
        # Identity is preloaded later, after c2's ACT, so its 1.3µs table
        # load runs while the PE does pool2/conv3.
        scratch = work.tile([1, 2], dt.float32)
        nc.vector.memset(scratch[:], 0.0)
        nc.scalar.activation(out=scratch[:], in_=scratch[:], func=AF.Lrelu,
                             bias=0.0, scale=1.0, alpha=0.01)

        def lrelu_bias(dst, src, bias_col):
            # dst = LeakyReLU(src + bias, slope 0.01) in one ACT op
            nc.scalar.activation(
                out=dst, in_=src, func=AF.Lrelu,
                bias=bt[:, bias_col:bias_col + 1], scale=1.0, alpha=0.01,
            )

        # --- conv1: 4 chunks of N=500/400 (chunk n = c1 rows 10n..10n+9).
        # Post-conv activations alternate Scalar ACT / Vector 2-op lrelu so
        # the four 500-elem activations run on two engines in parallel. ---
        rb = [0, 500, 1000, 1500, 1900]
        c1 = work.tile([128, 38, 50], dt.float16)
        c1f = c1[:].rearrange("p a b -> p (a b)")
        for n in range(4):
            sz = rb[n + 1] - rb[n]
            pc = ps.tile([128, 500], dt.float32, tag="ps")
            nc.tensor.matmul(pc[:, 0:sz], w1t, r1t[:, rb[n]:rb[n + 1]],
                             start=True, stop=True)
            lrelu_bias(c1f[:, rb[n]:rb[n + 1]], pc[:, 0:sz], 0)

        # Bridge warm-up: keep the PE busy between conv1 and conv2 so HAM
        # stays on its way to 2.4 GHz instead of re-throttling.
        for _ in range(2):
            nc.tensor.matmul(pw[:], warm[:, 0:128], warm[:], start=True, stop=True)

        # --- pool1: per-chunk row-pair max (pipelines under conv1), then one
        # column-pair max -> P1 (75->no, 128 partitions, 19, 25) fp16 ---
        rm1 = work.tile([128, 19, 50], dt.float16)
        for n in range(4):
            p0, p1_ = 5 * n, 5 * n + (5 if n < 3 else 4)
            nc.vector.tensor_max(out=rm1[:, p0:p1_],
                                 in0=c1[:, 2 * p0:2 * p1_:2, :],
                                 in1=c1[:, 2 * p0 + 1:2 * p1_:2, :])
        # colmax split at row 15: conv2's dy=0 taps only read P1 rows 0-14,
        # so they start (region deps) while rows 15-18 still finish.
        P1 = work.tile([128, 19, 25], dt.float16)
        nc.vector.tensor_max(out=P1[:, 0:15], in0=rm1[:, 0:15, 0:49:2],
                             in1=rm1[:, 0:15, 1:50:2])
        nc.vector.tensor_max(out=P1[:, 15:19], in0=rm1[:, 15:19, 0:49:2],
                             in1=rm1[:, 15:19, 1:50:2])

        # --- conv2: 25 accumulating matmuls, N=15x21=315 ---
        p2 = ps.tile([128, 15, 21], dt.float32, tag="ps")
        for dy in range(5):
            for dx in range(5):
                t = dy * 5 + dx
                nc.tensor.matmul(p2[:], w2t[:, t, :], P1[:, dy:dy + 15, dx:dx + 21],
                                 start=(t == 0), stop=(t == 24))
        # c2 activation in two row-halves so pool2's row-max pipeline starts
        # while the second half still activates.
        c2 = work.tile([128, 15, 21], dt.float16)
        lrelu_bias(c2[:, 0:8], p2[:, 0:8], 1)
        lrelu_bias(c2[:, 8:15], p2[:, 8:15], 1)

        # Keep the PE busy through the ACT+pool2 stall so HAM stays warm.
        for _ in range(3):
            nc.tensor.matmul(pw[:], warm[:, 0:128], warm[:], start=True, stop=True)

        # Preload the Identity ACT table while pool2/conv3 run.
        nc.scalar.activation(out=scratch[:], in_=scratch[:], func=AF.Identity,
                             bias=0.0, scale=1.0)

        # --- pool2: row-pair max then col-pair max on the full map, then
        # 4 strided gathers (vector + gpsimd in parallel) into combo-major
        # P2. cm2[y, x] = max over c2[{y,y+1},{x,x+1}]; combo (py,px) map
        # is cm2[py::2, px::2]. ---
        rm2 = work.tile([128, 14, 21], dt.float16)
        nc.vector.tensor_max(out=rm2[:, 0:7], in0=c2[:, 0:7, :], in1=c2[:, 1:8, :])
        nc.vector.tensor_max(out=rm2[:, 7:14], in0=c2[:, 7:14, :],
                             in1=c2[:, 8:15, :])
        cm2 = work.tile([128, 14, 20], dt.float16)
        nc.vector.tensor_max(out=cm2[:], in0=rm2[:, :, 0:20], in1=rm2[:, :, 1:21])
        P2 = work.tile([128, 4, 7, 10], dt.float16)
        for py in range(2):
            for px in range(2):
                eng = nc.vector if px == 0 else nc.gpsimd
                eng.tensor_copy(out=P2[:, py * 2 + px],
                                in_=cm2[:, py:py + 13:2, px:px + 19:2])

        # --- conv3: 25 accumulating matmuls, N=72 = (combo 4, i 3, j 6) ---
        p3 = ps.tile([128, 72], dt.float32, tag="ps")
        p3v = p3[:].rearrange("p (c i j) -> p c i j", c=4, i=3)
        for e in range(5):
            for f in range(5):
                t = e * 5 + f
                nc.tensor.matmul(p3v, w3t[:, t, :], P2[:, :, e:e + 3, f:f + 6],
                                 start=(t == 0), stop=(t == 24))
        h3 = work.tile([128, 72], dt.float16)
        lrelu_bias(h3[:], p3[:], 2)

        # --- conv4 (2 output halves): half0 ACT on Scalar, half1 on Vector ---
        h4 = work.tile([128, 2, 72], dt.float16)
        p4a = ps.tile([128, 72], dt.float32, tag="ps")
        p4b = ps.tile([128, 72], dt.float32, tag="ps")
        nc.tensor.matmul(p4a[:], w45t[:, 0, :], h3[:], start=True, stop=True)
        nc.tensor.matmul(p4b[:], w45t[:, 1, :], h3[:], start=True, stop=True)
        lrelu_bias(h4[:, 0], p4a[:], 3)
        t4 = work.tile([128, 72], dt.float32)
        nc.vector.tensor_scalar_add(t4[:], p4b[:], bt[:, 4:5])
        nc.vector.scalar_tensor_tensor(h4[:, 1], t4[:], 0.01, t4[:],
                                       op0=ALU.mult, op1=ALU.max)

        # --- conv5 (accumulate 2 K-halves) ---
        p5 = ps.tile([128, 72], dt.float32, tag="ps")
        nc.tensor.matmul(p5[:], w45t[:, 2, :], h4[:, 0], start=True, stop=False)
        nc.tensor.matmul(p5[:], w45t[:, 3, :], h4[:, 1], start=False, stop=True)
        h5 = work.tile([128, 72], dt.float16)
        lrelu_bias(h5[:], p5[:], 5)

        # --- dense (4 output quarters), bias only: 2 on Scalar, 2 on Vector ---
        out_t = work.tile([128, 4, 72], dt.float16)
        pd = [ps.tile([128, 72], dt.float32, tag="ps", name=f"pd{q}")
              for q in range(4)]
        for q in range(4):
            nc.tensor.matmul(pd[q][:], w45t[:, 4 + q, :], h5[:],
                             start=True, stop=True)
        for q in range(4):
            if q % 2 == 0:
                nc.scalar.activation(out=out_t[:, q], in_=pd[q][:],
                                     func=AF.Identity,
                                     bias=bt[:, 6 + q:7 + q], scale=1.0)
            else:
                nc.vector.tensor_scalar_add(out_t[:, q], pd[q][:],
                                            bt[:, 6 + q:7 + q])
        nc.sync.dma_start(out=FEATS, in_=out_t[:].rearrange("p q n -> p (q n)"))
    nc.compile()
    return nc


def _get_nc():
    if "nc" not in _CACHE:
        _CACHE["nc"] = _build_nc()
    return _CACHE["nc"]


def _run(in_maps, trace=False):
    from concourse.bass_utils import run_bass_kernel_spmd
    return run_bass_kernel_spmd(_get_nc(), in_maps, core_ids=list(range(8)),
                                trace=trace)


def _assemble(feats_list):
    out = np.zeros((1, 512, IMH, IMW), np.float32)
    ii = np.arange(3)
    jj = np.arange(6)
    for c in range(8):
        oy, ox, h = (c >> 2) & 1, (c >> 1) & 1, c & 1
        f = (feats_list[c].astype(np.float32).reshape(128, 4, 72)
             .transpose(1, 0, 2).reshape(512, 4, 3, 6))
        for py in range(2):
            for px in range(2):
                i_idx = 4 * (3 * h + ii) + 2 * py + oy
                j_idx = 4 * jj + 2 * px + ox
                out[0, :, i_idx[:, None], j_idx[None, :]] = (
                    f[:, py * 2 + px].transpose(1, 2, 0)
                )
    return out


def kernel(**inputs):
    r1s, w1, w2, w3, w45d, biases = _host_prep(**inputs)
    in_maps = [
        {"r1": r1s[c], "w2": w2, "w3": w3, "biases": biases}
        for c in range(8)
    ]
    res = _run(in_maps)
    feats_list = [res.results[c]["feats"] for c in range(8)]
    return _assemble(feats_list)
